# revision 11
# baseline (speedup 1.0000x reference)
"""Tree-GRU (arity-8, depth-5) over embedded leaves on 8 TRN2 NeuronCores.

Sharding: data-parallel over subtrees. Each core takes 4096 contiguous leaves
and runs levels 5 and 4 of the tree locally (512 -> 64 parents). The last two
per-core levels (64 -> 8 -> 1) and the root are small latency-bound GRU
cascades (free dim <= 8) done on host in fp64 after gathering the per-core
level-4 outputs, mirroring the baseline's host-side root reduction.

Device layout is feature-transposed: tensors live as [128 part, 3 ktile, ...]
with feature f = 128*k + p, so the GRU matmuls contract the partition dim.

Embedding rows are fetched with dma_gather(transpose=True): the tokens are
host-permuted into child-major order so each of 8 gathers (one per GRU step,
512 rows) lands feature-major in SBUF, eliminating the PE transposes and
letting step 0 start after a single ~1.3us gather instead of 16 indirect DMAs.

Level 512 runs one PSUM bank per (role, jo) output tile at N=512: each step
emits the input-side (gi) matmuls of all three jo units before any
recurrent-side (hh) matmuls, so the tensor engine always has ~5.7us of
h-independent work buffered to hide the gate chain of the previous step.
Biases ride the scalar-activation bias port. The per-step output accumulator
is kept child-major so the final step writes level 4's input directly as a
fused raw-sum add (the 1/8 output-mean scale is folded into a pre-scaled
copy of W_ih used by level 4).
"""

import numpy as np
import ml_dtypes

ARITY = 8
DIM = 384
VOCAB = 32000
NCORES = 8
P = 128
J = 3  # DIM // 128 feature tiles
N_LEAVES = 32768
LEAVES_CORE = N_LEAVES // NCORES  # 4096
P5 = LEAVES_CORE // ARITY  # 512 level-5 parents per core
P4 = P5 // ARITY  # 64 level-4 parents per core

BF16 = ml_dtypes.bfloat16

_PROG_CACHE = {}


def _emit(tc, nc, aps):
    import concourse.mybir as mybir

    f32 = mybir.dt.float32
    bf16 = mybir.dt.bfloat16
    Sig = mybir.ActivationFunctionType.Sigmoid
    Tanh = mybir.ActivationFunctionType.Tanh
    Add = mybir.AluOpType.add
    Sub = mybir.AluOpType.subtract
    Mult = mybir.AluOpType.mult

    idxs, embed, wih_t, wih_s, whh_t, biases, out_hacc, out_hf = aps

    from contextlib import ExitStack

    with ExitStack() as ctx:
        const = ctx.enter_context(tc.tile_pool(name="const", bufs=1))
        xpool = ctx.enter_context(tc.tile_pool(name="xpool", bufs=1))
        state = ctx.enter_context(tc.tile_pool(name="state", bufs=1))
        gates = ctx.enter_context(tc.tile_pool(name="gates", bufs=4))
        pspool = ctx.enter_context(tc.tile_pool(name="pspool", bufs=4, space="PSUM"))
        pspool2 = ctx.enter_context(tc.tile_pool(name="pspool2", bufs=4, space="PSUM"))

        # ---- tokens (pre-permuted child-major int16), then per-child gathers ----
        idx_sb = const.tile([P, ARITY * (P5 // 16)], mybir.dt.int16)
        nc.sync.dma_start(idx_sb[:], idxs[:])

        x5 = xpool.tile([P, ARITY, J, P5], bf16, name="x5", tag="x5")
        ncols = P5 // 16
        for t in range(ARITY):
            c = ARITY - 1 - t  # children consumed in reverse: child 7 first
            nc.gpsimd.dma_gather(
                x5[:, c],
                embed[:],
                idx_sb[:, c * ncols : (c + 1) * ncols],
                P5,
                P5,
                DIM,
                transpose=True,
                queue_num=t % 2,
            )

        # ---- constants / weights ----
        wih_sb = const.tile([P, J, 9, P], bf16)
        wih_s_sb = const.tile([P, J, 9, P], bf16)
        whh_sb = const.tile([P, J, 9, P], bf16)
        bias_sb = const.tile([P, 12], f32)
        nc.sync.dma_start(wih_sb[:], wih_t[:])
        nc.sync.dma_start(wih_s_sb[:], wih_s[:])
        nc.sync.dma_start(whh_sb[:], whh_t[:])
        nc.sync.dma_start(bias_sb[:], biases[:])

        x4 = xpool.tile([P, ARITY, J, P4], bf16, name="x4", tag="x4")

        def psum_tile(jo):
            # 8 banks for 12 role-tiles per step: unit j1 owns 4 banks
            # (step-to-step double... reuse waits on last step's j1 gates);
            # units j0 and j2 share the other 4 — j2's allocation waits on
            # same-step j0 gates, which fire progressively (ps_r first) while
            # hh j1 keeps the tensor engine busy, and j0's next-step
            # allocation waits on j2's gates likewise. All waits point at
            # strictly earlier FIFO positions, so no deadlock.
            if jo == 1:
                return pspool.tile([P, 512], f32, name="ps", tag="ps")
            return pspool2.tile([P, 512], f32, name="ps2", tag="ps2")

        # =================== level 5: 512 parents, leaf children ===================
        h5 = state.tile([P, J, P5], bf16, name="h5", tag="h5")
        hacc5 = state.tile([P, J, ARITY, P4], f32, name="hacc5", tag="hacc5")
        nc.gpsimd.memset(hacc5[:], 0.0)
        csum5 = state.tile([P, J, P4], f32, name="csum5", tag="csum5")

        with nc.named_scope("level_512"):
            for t in range(ARITY):
                c = ARITY - 1 - t
                leaf0 = t == 0

                ps_r, ps_z, ps_in, ps_hn = [None] * J, [None] * J, [None] * J, [None] * J

                def emit_gi(jo):
                    ps_r[jo] = psum_tile(jo)
                    ps_z[jo] = psum_tile(jo)
                    ps_in[jo] = psum_tile(jo)
                    if not leaf0:
                        ps_hn[jo] = psum_tile(jo)
                    for ps, moff in ((ps_r[jo], 0), (ps_z[jo], 3), (ps_in[jo], 6)):
                        for ji in range(J):
                            nc.tensor.matmul(
                                ps[:, :P5],
                                wih_sb[:, ji, moff + jo, :],
                                x5[:, c, ji, :],
                                start=(ji == 0),
                                stop=(ji == 2 and (moff == 6 or leaf0)),
                            )

                def emit_hh(jo):
                    if leaf0:
                        return
                    for ps, moff in ((ps_r[jo], 0), (ps_z[jo], 3), (ps_hn[jo], 6)):
                        for ji in range(J):
                            nc.tensor.matmul(
                                ps[:, :P5],
                                whh_sb[:, ji, moff + jo, :],
                                h5[:, ji, :],
                                start=(ji == 0 and moff == 6),
                                stop=(ji == 2),
                            )

                # gi of j0/j1 buffer ~3.8us of h-independent work ahead of the
                # first hh, hiding the previous step's gate chain; j2's psum
                # allocation then rides behind hh j1 while j0's gates free its
                # banks progressively.
                emit_gi(0)
                emit_gi(1)
                emit_hh(0)
                emit_hh(1)
                emit_gi(2)
                emit_hh(2)

                # --- gates per unit ---
                for jo in range(J):
                    r_sb = gates.tile([P, P5], bf16, name="r_sb", tag="r_sb")
                    z_sb = gates.tile([P, P5], bf16, name="z_sb", tag="z_sb")
                    n_sb = gates.tile([P, P5], bf16, name="n_sb", tag="n_sb")
                    rhn = gates.tile([P, P5], f32, name="rhn", tag="rhn")
                    t1 = gates.tile([P, P5], bf16, name="t1", tag="t1")

                    nc.scalar.activation(
                        r_sb[:], ps_r[jo][:, :P5], Sig, bias=bias_sb[:, jo : jo + 1]
                    )
                    nc.scalar.activation(
                        z_sb[:], ps_z[jo][:, :P5], Sig, bias=bias_sb[:, 3 + jo : 4 + jo]
                    )
                    if leaf0:
                        nc.vector.tensor_scalar_mul(
                            rhn[:], r_sb[:], bias_sb[:, 6 + jo : 7 + jo]
                        )
                    else:
                        nc.vector.scalar_tensor_tensor(
                            out=rhn[:],
                            in0=ps_hn[jo][:, :P5],
                            scalar=bias_sb[:, 6 + jo : 7 + jo],
                            in1=r_sb[:],
                            op0=Add,
                            op1=Mult,
                        )
                    nc.vector.tensor_tensor(
                        out=rhn[:], in0=rhn[:], in1=ps_in[jo][:, :P5], op=Add
                    )
                    nc.scalar.activation(
                        n_sb[:], rhn[:], Tanh, bias=bias_sb[:, 9 + jo : 10 + jo]
                    )
                    hsl = h5[:, jo, :]
                    if leaf0:
                        # h = (1-z)*n with h_prev = 0
                        nc.vector.tensor_tensor(out=t1[:], in0=z_sb[:], in1=n_sb[:], op=Mult)
                        nc.vector.tensor_tensor(out=hsl, in0=n_sb[:], in1=t1[:], op=Sub)
                    else:
                        nc.vector.tensor_tensor(out=t1[:], in0=hsl, in1=n_sb[:], op=Sub)
                        nc.vector.tensor_tensor(out=t1[:], in0=z_sb[:], in1=t1[:], op=Mult)
                        nc.vector.tensor_tensor(out=hsl, in0=n_sb[:], in1=t1[:], op=Add)

                    hperm = hsl.rearrange("p (q c) -> p c q", c=ARITY)
                    if t == ARITY - 1:
                        # child-mean of final hiddens -> h0 of level 4
                        nc.vector.tensor_reduce(
                            out=csum5[:, jo, :],
                            in_=hsl.rearrange("p (q c) -> p q c", c=ARITY),
                            axis=mybir.AxisListType.X,
                            op=Add,
                        )
                        # x4 = hacc + h (raw sum; /8 folded into wih_s at level 4)
                        nc.gpsimd.tensor_tensor(
                            out=x4[:, :, jo, :],
                            in0=hacc5[:, jo],
                            in1=hperm,
                            op=Add,
                        )
                    else:
                        nc.gpsimd.tensor_tensor(
                            out=hacc5[:, jo], in0=hacc5[:, jo], in1=hperm, op=Add
                        )

        # =================== level 4: 64 parents ===================
        h4 = state.tile([P, J, P4], bf16, name="h4", tag="h4")
        nc.scalar.mul(h4[:], csum5[:], 1.0 / ARITY)
        hacc4 = state.tile([P, J, P4], f32, name="hacc4", tag="hacc4")
        nc.gpsimd.memset(hacc4[:], 0.0)

        with nc.named_scope("level_64"):
            for t in range(ARITY):
                c = ARITY - 1 - t
                ps_r, ps_z, ps_in, ps_hn = [None] * J, [None] * J, [None] * J, [None] * J

                def emit_gi4(jo):
                    ps_r[jo] = psum_tile(jo)
                    ps_z[jo] = psum_tile(jo)
                    ps_in[jo] = psum_tile(jo)
                    ps_hn[jo] = psum_tile(jo)
                    for ps, moff in ((ps_r[jo], 0), (ps_z[jo], 3), (ps_in[jo], 6)):
                        for ji in range(J):
                            nc.tensor.matmul(
                                ps[:, :P4],
                                wih_s_sb[:, ji, moff + jo, :],
                                x4[:, c, ji, :],
                                start=(ji == 0),
                                stop=(ji == 2 and moff == 6),
                            )

                def emit_hh4(jo):
                    for ps, moff in ((ps_r[jo], 0), (ps_z[jo], 3), (ps_hn[jo], 6)):
                        for ji in range(J):
                            nc.tensor.matmul(
                                ps[:, :P4],
                                whh_sb[:, ji, moff + jo, :],
                                h4[:, ji, :],
                                start=(ji == 0 and moff == 6),
                                stop=(ji == 2),
                            )

                emit_gi4(0)
                emit_gi4(1)
                emit_hh4(0)
                emit_hh4(1)
                emit_gi4(2)
                emit_hh4(2)

                for jo in range(J):
                    r_sb = gates.tile([P, P4], bf16, name="r4", tag="r4")
                    z_sb = gates.tile([P, P4], bf16, name="z4", tag="z4")
                    n_sb = gates.tile([P, P4], bf16, name="n4", tag="n4")
                    rhn = gates.tile([P, P4], f32, name="rhn4", tag="rhn4")
                    t1 = gates.tile([P, P4], bf16, name="t14", tag="t14")

                    nc.scalar.activation(
                        r_sb[:], ps_r[jo][:, :P4], Sig, bias=bias_sb[:, jo : jo + 1]
                    )
                    nc.scalar.activation(
                        z_sb[:], ps_z[jo][:, :P4], Sig, bias=bias_sb[:, 3 + jo : 4 + jo]
                    )
                    nc.vector.scalar_tensor_tensor(
                        out=rhn[:],
                        in0=ps_hn[jo][:, :P4],
                        scalar=bias_sb[:, 6 + jo : 7 + jo],
                        in1=r_sb[:],
                        op0=Add,
                        op1=Mult,
                    )
                    nc.vector.tensor_tensor(
                        out=rhn[:], in0=rhn[:], in1=ps_in[jo][:, :P4], op=Add
                    )
                    nc.scalar.activation(
                        n_sb[:], rhn[:], Tanh, bias=bias_sb[:, 9 + jo : 10 + jo]
                    )
                    hsl = h4[:, jo, :]
                    nc.vector.tensor_tensor(out=t1[:], in0=hsl, in1=n_sb[:], op=Sub)
                    nc.vector.tensor_tensor(out=t1[:], in0=z_sb[:], in1=t1[:], op=Mult)
                    nc.vector.tensor_tensor(out=hsl, in0=n_sb[:], in1=t1[:], op=Add)
                    nc.gpsimd.tensor_tensor(
                        out=hacc4[:, jo], in0=hacc4[:, jo], in1=hsl, op=Add
                    )

        # ---- outputs: raw h-sum (x3*8) and final hiddens of the 64 nodes ----
        nc.sync.dma_start(out_hacc[:], hacc4[:])
        nc.sync.dma_start(out_hf[:], h4[:])


def _build_program():
    if "prog" in _PROG_CACHE:
        return _PROG_CACHE["prog"]
    import concourse.bacc as bacc
    import concourse.mybir as mybir
    import concourse.tile as tile

    f32 = mybir.dt.float32
    bf16 = mybir.dt.bfloat16

    nc = bacc.Bacc(
        "TRN2",
        target_bir_lowering=False,
        debug=False,
        enable_asserts=False,
        num_devices=NCORES,
        num_swdge_queues=2,
    )
    idxs = nc.dram_tensor(
        "idxs", [P, ARITY * (P5 // 16)], mybir.dt.int16, kind="ExternalInput"
    ).ap()
    embed = nc.dram_tensor("embed", [VOCAB, DIM], bf16, kind="ExternalInput").ap()
    wih_t = nc.dram_tensor("wih_t", [P, J, 9, P], bf16, kind="ExternalInput").ap()
    wih_s = nc.dram_tensor("wih_s", [P, J, 9, P], bf16, kind="ExternalInput").ap()
    whh_t = nc.dram_tensor("whh_t", [P, J, 9, P], bf16, kind="ExternalInput").ap()
    biases = nc.dram_tensor("biases", [P, 12], f32, kind="ExternalInput").ap()
    out_hacc = nc.dram_tensor("out_hacc", [P, J, P4], f32, kind="ExternalOutput").ap()
    out_hf = nc.dram_tensor("out_hf", [P, J, P4], bf16, kind="ExternalOutput").ap()

    with tile.TileContext(nc) as tc:
        _emit(tc, nc, (idxs, embed, wih_t, wih_s, whh_t, biases, out_hacc, out_hf))
    nc.compile()
    _PROG_CACHE["prog"] = nc
    return nc


def _retile_weights(w):
    # w: [1152, 384] -> lhsT tiles [128(k_part), 3(k), 9(m), 128(m_col)] bf16
    wt = np.ascontiguousarray(w.T)  # [384, 1152]
    wt = wt.reshape(J, P, 9, P).transpose(1, 0, 2, 3)
    return np.ascontiguousarray(wt).astype(BF16)


def _prep_bias(b_ih, b_hh):
    biases = np.zeros((P, 12), np.float32)
    comb = (b_ih + b_hh).reshape(9, P)
    biases[:, 0:6] = comb[0:6].T
    biases[:, 6:9] = b_hh.reshape(9, P)[6:9].T
    biases[:, 9:12] = b_ih.reshape(9, P)[6:9].T
    return biases


def _prep_idxs(tokens_core):
    # child-major gather order: position (c, q) holds tokens[q*8 + c], wrapped
    # into 16 partitions ([16, n/16] col-major-of-16) and replicated x8.
    tok = tokens_core.reshape(P5, ARITY).T  # [8 child, 512 parent]
    out = np.empty((16, ARITY * (P5 // 16)), np.int16)
    ncols = P5 // 16
    for c in range(ARITY):
        out[:, c * ncols : (c + 1) * ncols] = (
            tok[c].astype(np.int16).reshape(ncols, 16).T
        )
    return np.ascontiguousarray(np.tile(out, (8, 1)))


def _gru_level(x_children, h0, w_ih, w_hh, b_ih, b_hh):
    # x_children: [A, N, D] in original child order; consumed reversed.
    h = h0
    acc = np.zeros_like(h)
    for t in range(ARITY):
        x_t = x_children[ARITY - 1 - t]
        gi = x_t @ w_ih.T + b_ih
        gh = h @ w_hh.T + b_hh
        i_r, i_z, i_n = np.split(gi, 3, axis=-1)
        h_r, h_z, h_n = np.split(gh, 3, axis=-1)
        r = 1.0 / (1.0 + np.exp(-(i_r + h_r)))
        z = 1.0 / (1.0 + np.exp(-(i_z + h_z)))
        n = np.tanh(i_n + r * h_n)
        h = (1.0 - z) * n + z * h
        acc += h
    return acc / ARITY, h


def kernel(leaf_tokens, embed_table, w_ih, w_hh, b_ih, b_hh):
    from concourse.bass_utils import run_bass_kernel_spmd

    leaf_tokens = np.asarray(leaf_tokens, np.int32)
    embed_table = np.asarray(embed_table, np.float32)
    w_ih = np.asarray(w_ih, np.float32)
    w_hh = np.asarray(w_hh, np.float32)
    b_ih = np.asarray(b_ih, np.float32)
    b_hh = np.asarray(b_hh, np.float32)

    nc = _build_program()

    embed_bf = embed_table.astype(BF16)
    wih_t = _retile_weights(w_ih)
    wih_s = _retile_weights(w_ih / ARITY)
    whh_t = _retile_weights(w_hh)
    biases = _prep_bias(b_ih, b_hh)
    in_maps = []
    for core in range(NCORES):
        in_maps.append(
            {
                "idxs": _prep_idxs(
                    leaf_tokens[core * LEAVES_CORE : (core + 1) * LEAVES_CORE]
                ),
                "embed": embed_bf,
                "wih_t": wih_t,
                "wih_s": wih_s,
                "whh_t": whh_t,
                "biases": biases,
            }
        )
    res = run_bass_kernel_spmd(nc, in_maps, core_ids=list(range(NCORES)))

    # device tensors -> [core, 64 nodes, 384] with f = j*128 + p
    x3 = np.zeros((NCORES, P4, DIM), np.float64)
    h3 = np.zeros((NCORES, P4, DIM), np.float64)
    for core in range(NCORES):
        hacc = np.asarray(res.results[core]["out_hacc"], np.float64)  # [128,3,64]
        hf = np.asarray(res.results[core]["out_hf"], np.float64)
        x3[core] = (hacc / ARITY).transpose(1, 0, 2).reshape(DIM, P4).T
        h3[core] = hf.transpose(1, 0, 2).reshape(DIM, P4).T

    w_ih64 = w_ih.astype(np.float64)
    w_hh64 = w_hh.astype(np.float64)
    b_ih64 = b_ih.astype(np.float64)
    b_hh64 = b_hh.astype(np.float64)

    # level 3: per core, 8 parents x 8 children (batch over cores*parents)
    xc = x3.reshape(NCORES * ARITY, ARITY, DIM).transpose(1, 0, 2)  # [A, 64, D]
    h0 = h3.reshape(NCORES * ARITY, ARITY, DIM).mean(axis=1)
    x2, h2 = _gru_level(xc, h0, w_ih64, w_hh64, b_ih64, b_hh64)

    # level 2: per core, 1 parent x 8 children
    xc = x2.reshape(NCORES, ARITY, DIM).transpose(1, 0, 2)  # [A, 8, D]
    h0 = h2.reshape(NCORES, ARITY, DIM).mean(axis=1)
    x1, h1 = _gru_level(xc, h0, w_ih64, w_hh64, b_ih64, b_hh64)

    # root: 8 cores' outputs
    xc = x1.reshape(1, ARITY, DIM).transpose(1, 0, 2)  # [A, 1, D]
    h0 = h1.reshape(1, ARITY, DIM).mean(axis=1)
    out, _ = _gru_level(xc, h0, w_ih64, w_hh64, b_ih64, b_hh64)

    return out.astype(np.float32).reshape(1, 1, DIM)
